# revision 2
# baseline (speedup 1.0000x reference)
"""OAdder2d_Q (oconv, 16-bit dorefa quant) as an 8-core Trainium2 Bass kernel.

Math: with ideal disks the op is a 3x3/pad1 conv with effective kernel
w_q * sin(phases)*(d0+d1)/2.  Input quantize (round(clip(x)*65535)/65535)
runs on-device; the tiny weight transform (tanh/dorefa + phase fold) runs
on host.  Data-parallel over batch: 32 images -> 4 per core, weights
replicated.  The conv is computed as 9 shifted matmuls (one per kernel tap)
accumulating in PSUM, operands in fp16 (exact to ~6e-4 scale-relative).
"""

import sys

if "/opt/trn_rl_repo" not in sys.path:
    sys.path.insert(0, "/opt/trn_rl_repo")

import numpy as np

import concourse.bacc as bacc
import concourse.mybir as mybir
from concourse.tile import TileContext
from concourse.bass_utils import run_bass_kernel_spmd

N_CORES = 8
B, C, O, K, H, W = 32, 128, 256, 3, 56, 56
PB = B // N_CORES              # images per core
HP, WP = H + 2, W + 2          # padded spatial
RB = 8                         # output rows per psum tile
NRB = H // RB                  # row blocks per image
NT = RB * W                    # moving/free elems per matmul (448)
QN = 65535.0                   # 2^16 - 1
MAGIC = float(2 ** 23)         # fp32 round-to-nearest-integer constant

f32 = mybir.dt.float32
f16 = mybir.dt.float16

_CACHE = {}


def _build_nc():
    nc = bacc.Bacc("TRN2", target_bir_lowering=False, debug=False,
                   num_devices=N_CORES)
    x = nc.dram_tensor("x", (PB, C, H, W), f32, kind="ExternalInput")
    w = nc.dram_tensor("w", (C, 9 * O), f16, kind="ExternalInput")
    y = nc.dram_tensor("y", (PB, O, H, W), f32, kind="ExternalOutput")

    # const AP for the activation bias (round-to-int magic constant)
    magic_t = nc.alloc_sbuf_tensor(f"const-float32-{MAGIC}", [128, 1], f32)
    nc.gpsimd.memset(magic_t.ap(), MAGIC)
    nc.const_aps.aps[(f32, MAGIC)] = magic_t.ap()

    with TileContext(nc) as tc:
        with tc.tile_pool(name="wp", bufs=1) as wp, \
             tc.tile_pool(name="xsp", bufs=2) as xsp, \
             tc.tile_pool(name="tp", bufs=2) as tp, \
             tc.tile_pool(name="xpp", bufs=2) as xpp, \
             tc.tile_pool(name="pp", bufs=6, space="PSUM") as pp, \
             tc.tile_pool(name="op", bufs=2) as outp:
            wt = wp.tile([C, 9 * O], f16)
            nc.sync.dma_start(out=wt, in_=w[:, :])
            for img in range(PB):
                xs = xsp.tile([C, H, W], f32)
                nc.sync.dma_start(out=xs, in_=x[img, :, :, :])
                # t = x*65535 + 2^23  (fp32 add of 2^23 == round-half-even
                # to integer, matching jnp.round; x is already in [0, 1))
                t = tp.tile([C, H, W], f32)
                nc.scalar.activation(out=t, in_=xs,
                                     func=mybir.ActivationFunctionType.Identity,
                                     bias=MAGIC, scale=QN)
                # x_q = (t - 2^23) / 65535, cast fp16, into padded tile
                xp = xpp.tile([C, HP, WP], f16)
                nc.vector.memset(xp[:, 0, :], 0.0)
                nc.vector.memset(xp[:, HP - 1, :], 0.0)
                nc.vector.memset(xp[:, 1:HP - 1, 0], 0.0)
                nc.vector.memset(xp[:, 1:HP - 1, WP - 1], 0.0)
                nc.vector.tensor_scalar(
                    out=xp[:, 1:H + 1, 1:W + 1], in0=t,
                    scalar1=MAGIC, scalar2=1.0 / QN,
                    op0=mybir.AluOpType.subtract, op1=mybir.AluOpType.mult)
                for oh in range(O // 128):
                    yt = outp.tile([128, H, W], f32)
                    for rb in range(NRB):
                        ps = pp.tile([128, RB, W], f32)
                        for ki in range(K):
                            for kj in range(K):
                                kidx = ki * K + kj
                                rhs = xp[:, rb * RB + ki: rb * RB + ki + RB,
                                         kj: kj + W]
                                lhsT = wt[:, kidx * O + oh * 128:
                                          kidx * O + oh * 128 + 128]
                                nc.tensor.matmul(ps, lhsT, rhs,
                                                 start=(kidx == 0),
                                                 stop=(kidx == K * K - 1))
                        dst = yt[:, rb * RB:(rb + 1) * RB, :]
                        if rb % 2 == 0:
                            nc.vector.tensor_copy(out=dst, in_=ps)
                        else:
                            nc.scalar.copy(out=dst, in_=ps)
                    nc.sync.dma_start(
                        out=y[img, oh * 128:(oh + 1) * 128, :, :], in_=yt)
    nc.compile()
    return nc


def _prep_weights(weight, phases, disks):
    """dorefa weight quantize + fold phases/disks into the conv kernel."""
    t = np.tanh(weight.astype(np.float32))
    t = t / (2.0 * np.max(np.abs(t))) + 0.5
    wq = (np.round(t * QN) / np.float32(QN)).astype(np.float32)
    s = np.sin(phases.astype(np.float32))[0, 0]        # (C,K,K)
    d0 = disks[0, 0, ..., 0].astype(np.float32)
    d1 = disks[0, 0, ..., 1].astype(np.float32)
    k_mul = wq * (s * (d0 + d1) * 0.5)[None]           # (O,C,K,K)
    # lhsT layout: [c, kidx*O + o]
    wsb = np.ascontiguousarray(
        k_mul.transpose(1, 2, 3, 0).reshape(C, 9 * O)).astype(np.float16)
    coef = (d0 - d1) * 0.25                            # (C,K,K)
    return wsb, wq, coef


def _square_terms(x, wq, coef):
    """Generic-disk correction (zero for ideal disks): conv(x_q^2, coef)
    broadcast over O, plus per-O constant sum(w_q^2 * coef)."""
    xq = np.round(np.clip(x, 0.0, 1.0) * QN) / np.float32(QN)
    x2 = (xq * xq).astype(np.float32)
    bsz = x.shape[0]
    x2p = np.zeros((bsz, C, H + 2, W + 2), np.float32)
    x2p[:, :, 1:H + 1, 1:W + 1] = x2
    y_sq = np.zeros((bsz, H, W), np.float32)
    for ki in range(K):
        for kj in range(K):
            y_sq += np.einsum("bchw,c->bhw",
                              x2p[:, :, ki:ki + H, kj:kj + W],
                              coef[:, ki, kj], optimize=True)
    w_term = np.einsum("ockk,ckk->o", wq * wq, coef)
    return y_sq[:, None] + w_term[None, :, None, None]


def kernel(x, weight, phases, disks):
    x = np.asarray(x)
    wsb, wq, coef = _prep_weights(np.asarray(weight), np.asarray(phases),
                                  np.asarray(disks))
    if "nc" not in _CACHE:
        _CACHE["nc"] = _build_nc()
    nc = _CACHE["nc"]
    in_maps = [{"x": np.ascontiguousarray(x[c * PB:(c + 1) * PB]), "w": wsb}
               for c in range(N_CORES)]
    res = run_bass_kernel_spmd(nc, in_maps, list(range(N_CORES)))
    y = np.concatenate([res.results[c]["y"] for c in range(N_CORES)], axis=0)
    if np.any(coef != 0.0):
        y = y + _square_terms(x, wq, coef)
    return y.astype(np.float32)


# revision 3
# speedup vs baseline: 1.0760x; 1.0760x over previous
"""OAdder2d_Q (oconv, 16-bit dorefa quant) as an 8-core Trainium2 Bass kernel.

Math: with ideal disks the op is a 3x3/pad1 conv with effective kernel
w_q * sin(phases)*(d0+d1)/2.  Input quantize (round(clip(x)*65535)/65535)
runs on-device; the tiny weight transform (tanh/dorefa + phase fold) runs
on host.  Data-parallel over batch: 32 images -> 4 per core, weights
replicated.  The conv is computed as 9 shifted matmuls (one per kernel tap)
accumulating in PSUM, operands in fp16 (exact to ~6e-4 scale-relative).
"""

import sys

if "/opt/trn_rl_repo" not in sys.path:
    sys.path.insert(0, "/opt/trn_rl_repo")

import numpy as np

import concourse.bacc as bacc
import concourse.mybir as mybir
from concourse.tile import TileContext
from concourse.bass_utils import run_bass_kernel_spmd

N_CORES = 8
B, C, O, K, H, W = 32, 128, 256, 3, 56, 56
PB = B // N_CORES              # images per core
HP, WP = H + 2, W + 2          # padded spatial
RB = 8                         # output rows per psum tile
NRB = H // RB                  # row blocks per image
NT = RB * W                    # moving/free elems per matmul (448)
QN = 65535.0                   # 2^16 - 1
MAGIC = float(2 ** 23)         # fp32 round-to-nearest-integer constant

f32 = mybir.dt.float32
f16 = mybir.dt.float16

_CACHE = {}


def _build_nc():
    nc = bacc.Bacc("TRN2", target_bir_lowering=False, debug=False,
                   num_devices=N_CORES)
    x = nc.dram_tensor("x", (PB, C, H, W), f32, kind="ExternalInput")
    w = nc.dram_tensor("w", (C, 9 * O), f16, kind="ExternalInput")
    y = nc.dram_tensor("y", (PB, O, H, W), f32, kind="ExternalOutput")

    # const AP for the activation bias (round-to-int magic constant)
    magic_t = nc.alloc_sbuf_tensor(f"const-float32-{MAGIC}", [128, 1], f32)
    nc.gpsimd.memset(magic_t.ap(), MAGIC)
    nc.const_aps.aps[(f32, MAGIC)] = magic_t.ap()

    NCH = 4                    # input dma/quantize chunks per image
    CHR = H // NCH             # rows per chunk (14)
    with TileContext(nc) as tc:
        with tc.tile_pool(name="wp", bufs=1) as wp, \
             tc.tile_pool(name="xsp", bufs=3) as xsp, \
             tc.tile_pool(name="tp", bufs=3) as tp, \
             tc.tile_pool(name="xpp", bufs=2) as xpp, \
             tc.tile_pool(name="pp", bufs=6, space="PSUM") as pp, \
             tc.tile_pool(name="wup", bufs=1, space="PSUM") as wup, \
             tc.tile_pool(name="op", bufs=4) as outp:
            wt = wp.tile([C, 9 * O], f16)
            nc.sync.dma_start(out=wt, in_=w[:, :])
            # PE warm-up: ~4us of dummy matmuls with no data deps so the
            # HAM clock gate is at 8/8 by the time real matmuls start.
            wu_in = wp.tile([C, 64], f16)
            nc.vector.memset(wu_in, 0.0)
            wu_ps = wup.tile([32, 64], f32)
            for _ in range(48):
                nc.tensor.matmul(wu_ps, wu_in[:, :32], wu_in[:, :64],
                                 start=True, stop=True)
            for img in range(PB):
                # chunked DMA + quantize so early row-blocks' matmuls can
                # start before the whole image is resident (subtile deps)
                xp = xpp.tile([C, HP, WP], f16)
                nc.vector.memset(xp[:, 0, :], 0.0)
                nc.vector.memset(xp[:, HP - 1, :], 0.0)
                nc.vector.memset(xp[:, 1:HP - 1, 0], 0.0)
                nc.vector.memset(xp[:, 1:HP - 1, WP - 1], 0.0)
                for ch in range(NCH):
                    r0 = ch * CHR
                    xs = xsp.tile([C, CHR, W], f32)
                    nc.sync.dma_start(out=xs, in_=x[img, :, r0:r0 + CHR, :])
                    # t = x*65535 + 2^23 (fp32 add of 2^23 == round-half-even
                    # to integer, matching jnp.round; x is already in [0,1))
                    t = tp.tile([C, CHR, W], f32)
                    nc.scalar.activation(
                        out=t, in_=xs,
                        func=mybir.ActivationFunctionType.Identity,
                        bias=MAGIC, scale=QN)
                    # x_q = (t - 2^23) / 65535, cast fp16, into padded tile
                    nc.vector.tensor_scalar(
                        out=xp[:, r0 + 1:r0 + CHR + 1, 1:W + 1], in0=t,
                        scalar1=MAGIC, scalar2=1.0 / QN,
                        op0=mybir.AluOpType.subtract, op1=mybir.AluOpType.mult)
                for oh in range(O // 128):
                    for rb in range(NRB):
                        ps = pp.tile([128, RB, W], f32)
                        for ki in range(K):
                            for kj in range(K):
                                kidx = ki * K + kj
                                rhs = xp[:, rb * RB + ki: rb * RB + ki + RB,
                                         kj: kj + W]
                                lhsT = wt[:, kidx * O + oh * 128:
                                          kidx * O + oh * 128 + 128]
                                nc.tensor.matmul(ps, lhsT, rhs,
                                                 start=(kidx == 0),
                                                 stop=(kidx == K * K - 1))
                        yt = outp.tile([128, RB, W], f32)
                        if rb % 2 == 0:
                            nc.vector.tensor_copy(out=yt, in_=ps)
                        else:
                            nc.scalar.copy(out=yt, in_=ps)
                        nc.sync.dma_start(
                            out=y[img, oh * 128:(oh + 1) * 128,
                                  rb * RB:(rb + 1) * RB, :],
                            in_=yt)
    nc.compile()
    return nc


def _prep_weights(weight, phases, disks):
    """dorefa weight quantize + fold phases/disks into the conv kernel."""
    t = np.tanh(weight.astype(np.float32))
    t = t / (2.0 * np.max(np.abs(t))) + 0.5
    wq = (np.round(t * QN) / np.float32(QN)).astype(np.float32)
    s = np.sin(phases.astype(np.float32))[0, 0]        # (C,K,K)
    d0 = disks[0, 0, ..., 0].astype(np.float32)
    d1 = disks[0, 0, ..., 1].astype(np.float32)
    k_mul = wq * (s * (d0 + d1) * 0.5)[None]           # (O,C,K,K)
    # lhsT layout: [c, kidx*O + o]
    wsb = np.ascontiguousarray(
        k_mul.transpose(1, 2, 3, 0).reshape(C, 9 * O)).astype(np.float16)
    coef = (d0 - d1) * 0.25                            # (C,K,K)
    return wsb, wq, coef


def _square_terms(x, wq, coef):
    """Generic-disk correction (zero for ideal disks): conv(x_q^2, coef)
    broadcast over O, plus per-O constant sum(w_q^2 * coef)."""
    xq = np.round(np.clip(x, 0.0, 1.0) * QN) / np.float32(QN)
    x2 = (xq * xq).astype(np.float32)
    bsz = x.shape[0]
    x2p = np.zeros((bsz, C, H + 2, W + 2), np.float32)
    x2p[:, :, 1:H + 1, 1:W + 1] = x2
    y_sq = np.zeros((bsz, H, W), np.float32)
    for ki in range(K):
        for kj in range(K):
            y_sq += np.einsum("bchw,c->bhw",
                              x2p[:, :, ki:ki + H, kj:kj + W],
                              coef[:, ki, kj], optimize=True)
    w_term = np.einsum("ockk,ckk->o", wq * wq, coef)
    return y_sq[:, None] + w_term[None, :, None, None]


def kernel(x, weight, phases, disks):
    x = np.asarray(x)
    wsb, wq, coef = _prep_weights(np.asarray(weight), np.asarray(phases),
                                  np.asarray(disks))
    if "nc" not in _CACHE:
        _CACHE["nc"] = _build_nc()
    nc = _CACHE["nc"]
    in_maps = [{"x": np.ascontiguousarray(x[c * PB:(c + 1) * PB]), "w": wsb}
               for c in range(N_CORES)]
    res = run_bass_kernel_spmd(nc, in_maps, list(range(N_CORES)))
    y = np.concatenate([res.results[c]["y"] for c in range(N_CORES)], axis=0)
    if np.any(coef != 0.0):
        y = y + _square_terms(x, wq, coef)
    return y.astype(np.float32)
